# revision 1
# baseline (speedup 1.0000x reference)
"""FCOS detection head on 8 Trainium2 NeuronCores (Bass/Tile).

Data parallel: batch 16 -> 2 images per core. Weights replicated.

Per-core compute layout:
  - channels on SBUF partitions (256 ch -> 2 tiles of 128), spatial flattened
    on the free dim, activations stored zero-padded (H+2)x(W+2) so a 3x3 conv
    is 2(ci) x 9(taps) = 18 accumulating matmuls per PSUM tile.
  - matmuls run as float32r (full-rate fp32 path on the PE).
  - bias+ReLU epilogues on the scalar engine (ACT) straight out of PSUM into
    the next layer's padded buffer.
  - head outputs (85 = 80 cls + 4 box + 1 ctr channels) are assembled
    channels-on-partitions, then PE-transposed per 128-position chunk into
    (positions, 85) and DMA'd to HBM.
"""

import numpy as np

import concourse.bacc as bacc
import concourse.bass as bass
import concourse.mybir as mybir
import concourse.tile as tile
from concourse.bass import ts
from concourse.bass_utils import run_bass_kernel_spmd
from concourse.masks import make_identity

F32 = mybir.dt.float32
N_CORES = 8
B_FULL = 16
BS = B_FULL // N_CORES  # images per core
C = 256
NCLS = 80
SDEPTH = 4
TAPS = [(ky, kx) for ky in range(3) for kx in range(3)]

# (H, W, rows-per-block for direct convs / Winograd stem, output base offset);
# p5 runs both images per layer and uses direct stem (spatial too small for
# N=512 Winograd matmuls)
LEVELS = [
    dict(H=64, W=64, R=8, Rw=16, base=0, img_groups=[[0], [1]]),
    dict(H=32, W=32, R=16, Rw=32, base=4096, img_groups=[[0], [1]]),
    dict(H=16, W=16, R=16, Rw=None, base=5120, img_groups=[[0, 1]]),
]
HW_TOTAL = 64 * 64 + 32 * 32 + 16 * 16  # 5376

# matmul/storage dtype for conv operands: float16 runs the PE at full rate
# (1 row/cycle, like bf16) with 10 mantissa bits; PSUM accumulation is fp32.
# (fp32 matmul is 4x slower; fp32r's fused weight-load path caps at 2 sync
# waits per instruction, which Tile-scheduled code exceeds.)
F16 = mybir.dt.float16


def _conv_block(nc, psum, src, wslices, y0, R, W, start_clear=True):
    """18 accumulating matmuls: psum[M, R*W] += sum_{ci,tap} w.T @ x_shifted.

    src: padded activation tile [128, 2, H+2, W+2]
    wslices: wslices[ci][tap] -> lhsT AP [128, M]
    """
    n_ci = len(wslices)
    for ci in range(n_ci):
        for t, (dy, dx) in enumerate(TAPS):
            rhs = src[:, ci, y0 + dy : y0 + dy + R, dx : dx + W]
            nc.tensor.matmul(
                psum[:],
                wslices[ci][t],
                rhs,
                start=(start_clear and ci == 0 and t == 0),
                stop=(ci == n_ci - 1 and t == len(TAPS) - 1),
            )


def _border_memset(nc, buf, H, W):
    # zero the 1-px padding border of a [128, 2, H+2, W+2] tile
    nc.gpsimd.memset(buf[:, :, 0, :], 0.0)
    nc.gpsimd.memset(buf[:, :, H + 1, :], 0.0)
    nc.gpsimd.memset(buf[:, :, 1 : H + 1, 0], 0.0)
    nc.gpsimd.memset(buf[:, :, 1 : H + 1, W + 1], 0.0)


def build_nc():
    # Bacc so finalize() runs the wait-legalization passes (matmul waits
    # move to ldweights / event-semaphore splits) that walrus codegen needs.
    nc = bacc.Bacc()

    # --- DRAM parameters (per-core views) ---
    x_dram = {}
    for i, lvl in enumerate(LEVELS):
        H, W = lvl["H"], lvl["W"]
        # host-side zero-padded to (H+2, W+2): one fully contiguous DMA per
        # (image, ci-tile) and no on-chip border memsets for the x buffer
        x_dram[i] = nc.declare_dram_parameter(
            f"x_l{i}", [BS, C, H + 2, W + 2], F16, isOutput=False
        )
    w_cls = nc.declare_dram_parameter("w_cls", [SDEPTH, 2, 128, 2 * 9 * 128], F16, isOutput=False)
    w_box = nc.declare_dram_parameter("w_box", [SDEPTH, 2, 128, 2 * 9 * 128], F16, isOutput=False)
    # Winograd F(2,3)-transformed stem weights: cols = (co_t, i4, dy3, co128)
    w_cls_w = nc.declare_dram_parameter("w_cls_w", [SDEPTH, 2, 128, 2 * 4 * 3 * 128], F16, isOutput=False)
    w_box_w = nc.declare_dram_parameter("w_box_w", [SDEPTH, 2, 128, 2 * 4 * 3 * 128], F16, isOutput=False)
    w_pcls = nc.declare_dram_parameter("w_pcls", [2, 128, 9 * NCLS], F16, isOutput=False)
    w_pbc = nc.declare_dram_parameter("w_pbc", [2, 128, 9 * 5], F16, isOutput=False)
    b_stem = nc.declare_dram_parameter("b_stem", [128, 2 * SDEPTH * 2], F32, isOutput=False)
    b_pred = nc.declare_dram_parameter("b_pred", [85, 1], F32, isOutput=False)
    out_dram = nc.declare_dram_parameter("out", [BS, HW_TOTAL, 85], F32, isOutput=True)

    with tile.TileContext(nc) as tc:
        with (
            tc.tile_pool(name="const", bufs=1) as const,
            tc.tile_pool(name="wp", bufs=3) as wp,
            tc.tile_pool(name="acts", bufs=1) as acts,
            tc.tile_pool(name="stage", bufs=1) as stage,
            tc.tile_pool(name="pp", bufs=1, space="PSUM") as pp,
        ):
            # constants
            ident = const.tile([128, 128], F32, name="ident")
            make_identity(nc, ident[:])
            bst = const.tile([128, 2, SDEPTH, 2, 1], F32, name="bst")
            nc.sync.dma_start(out=bst[:, :, :, :, 0], in_=b_stem[:].rearrange("p (t l c) -> p t l c", t=2, l=SDEPTH, c=2))
            bp_cls = const.tile([NCLS, 1], F32, name="bp_cls")
            nc.sync.dma_start(out=bp_cls[:], in_=b_pred[0:NCLS])
            bp_bc = const.tile([5, 1], F32, name="bp_bc")
            nc.sync.dma_start(out=bp_bc[:], in_=b_pred[NCLS : NCLS + 5])
            wpc = const.tile([128, 2, 9 * NCLS], F16, name="wpc")
            wpb = const.tile([128, 2, 9 * 5], F16, name="wpb")
            for t in range(2):
                nc.sync.dma_start(out=wpc[:, t, :], in_=w_pcls[t])
                nc.sync.dma_start(out=wpb[:, t, :], in_=w_pbc[t])

            for li, lvl in enumerate(LEVELS):
                H, W, R, base = lvl["H"], lvl["W"], lvl["R"], lvl["base"]
                HP, WP = H + 2, W + 2
                nblk = H // R
                N = R * W  # psum free size per block

                for imgs in lvl["img_groups"]:
                    # padded activation buffers per image: x, A, B
                    xb, ab, bb, sbc = {}, {}, {}, {}
                    for slot, g in enumerate(imgs):
                        xb[g] = acts.tile([128, 2, HP, WP], F16, name=f"xb{slot}", tag=f"xb{slot}")
                        ab[g] = acts.tile([128, 2, HP, WP], F16, name=f"ab{slot}", tag=f"ab{slot}")
                        bb[g] = acts.tile([128, 2, HP, WP], F16, name=f"bb{slot}", tag=f"bb{slot}")
                        sbc[g] = stage.tile([NCLS, H * W], F32, name=f"sbc{slot}", tag=f"sbc{slot}")
                        for buf in (ab[g], bb[g]):
                            _border_memset(nc, buf, H, W)
                        hh = HP // 2
                        for t in range(2):
                            nc.sync.dma_start(
                                out=xb[g][:, t, 0:hh, :],
                                in_=x_dram[li][g, ts(t, 128), 0:hh],
                            )
                            nc.sync.dma_start(
                                out=xb[g][:, t, hh:HP, :],
                                in_=x_dram[li][g, ts(t, 128), hh:HP],
                            )

                    Rw = lvl["Rw"]
                    Wh = W // 2

                    def stem_layer_direct(tower_w, tower_idx, lay, src_of, dst_of):
                        wt = wp.tile([128, 2, 2 * 9 * 128], F16, name="wt", tag="wt")
                        for t in range(2):
                            nc.sync.dma_start(out=wt[:, t, :], in_=tower_w[lay, t])
                        for g in imgs:
                            src, dst = src_of[g], dst_of[g]
                            for blk in range(nblk):
                                y0 = blk * R
                                for co in range(2):
                                    ps = pp.tile([128, N], F32, name="ps", tag="ws0", bufs=2)
                                    wsl_co = [
                                        [wt[:, ci, ts(co * 9 + t, 128)] for t in range(9)]
                                        for ci in range(2)
                                    ]
                                    _conv_block(nc, ps, src, wsl_co, y0, R, W)
                                    nc.scalar.activation(
                                        dst[:, co, 1 + y0 : 1 + y0 + R, 1 : 1 + W],
                                        ps[:].rearrange("p (r w) -> p r w", w=W),
                                        mybir.ActivationFunctionType.Relu,
                                        bias=bst[:, tower_idx, lay, co, :],
                                    )

                    def stem_layer_wino(tower_w, tower_idx, lay, src_of, dst_of):
                        """1D Winograd F(2,3) along W: 24 matmuls of N=Rw*W/2
                        per (block, co) instead of 18 of N=Rw*W/... (1.5x fewer
                        PE rows). Input/output transforms run on the DVE."""
                        wt = wp.tile([128, 2, 2 * 4 * 3 * 128], F16, name="wtw", tag="wt")
                        for t in range(2):
                            nc.sync.dma_start(out=wt[:, t, :], in_=tower_w[lay, t])
                        add = mybir.AluOpType.add
                        sub = mybir.AluOpType.subtract
                        for g in imgs:
                            src, dst = src_of[g], dst_of[g]
                            for blk in range(H // Rw):
                                y0 = blk * Rw
                                # one tile per transform index so matmul group i
                                # only waits on its own V op (Tile deps are
                                # whole-tile); i-major emission gets the PE
                                # started after the first two ops.
                                vt = [
                                    stage.tile(
                                        [128, 2, Rw + 2, Wh], F16, name=f"vt{i}", tag=f"vt{i}", bufs=3
                                    )
                                    for i in range(4)
                                ]
                                vdef = [
                                    (0, 0, 2, sub),
                                    (1, 1, 2, add),
                                    (2, 2, 1, sub),
                                    (3, 1, 3, sub),
                                ]
                                for i, a, b, op in vdef:
                                    for ci in range(2):
                                        rows = src[:, ci, y0 : y0 + Rw + 2, :]
                                        nc.vector.tensor_tensor(
                                            vt[i][:, ci],
                                            rows[:, :, a : a + W - 1 : 2],
                                            rows[:, :, b : b + W - 1 : 2],
                                            op,
                                        )
                                for co in range(2):
                                    ps = [
                                        pp.tile([128, Rw, Wh], F32, name=f"ws{i}", tag=f"ws{i}", bufs=2)
                                        for i in range(4)
                                    ]
                                    for i in range(4):
                                        for dy in range(3):
                                            for ci in range(2):
                                                nc.tensor.matmul(
                                                    ps[i][:],
                                                    wt[:, ci, ts((co * 4 + i) * 3 + dy, 128)],
                                                    vt[i][:, ci, dy : dy + Rw, :],
                                                    start=(dy == 0 and ci == 0),
                                                    stop=(dy == 2 and ci == 1),
                                                )
                                    # DVE may read only ONE PSUM operand per op:
                                    # stage m2 in SBUF (on ACT), then combine on
                                    # DVE with one PSUM operand per instruction.
                                    c2 = stage.tile([128, Rw, Wh], F32, name="c2", tag="c2", bufs=2)
                                    t0 = stage.tile([128, Rw, Wh], F32, name="t0", tag="t0", bufs=2)
                                    e0 = stage.tile([128, Rw, Wh], F32, name="e0", tag="e0", bufs=2)
                                    e1 = stage.tile([128, Rw, Wh], F32, name="e1", tag="e1", bufs=2)
                                    nc.scalar.activation(
                                        c2[:], ps[2][:], mybir.ActivationFunctionType.Copy
                                    )
                                    nc.vector.tensor_tensor(t0[:], ps[1][:], c2[:], add)
                                    nc.vector.tensor_tensor(e0[:], ps[0][:], t0[:], add)
                                    nc.vector.tensor_tensor(e1[:], ps[1][:], c2[:], sub)
                                    nc.vector.tensor_tensor(e1[:], e1[:], ps[3][:], sub)
                                    nc.scalar.activation(
                                        dst[:, co, 1 + y0 : 1 + y0 + Rw, 1 : W + 1 : 2],
                                        e0[:],
                                        mybir.ActivationFunctionType.Relu,
                                        bias=bst[:, tower_idx, lay, co, :],
                                    )
                                    nc.scalar.activation(
                                        dst[:, co, 1 + y0 : 1 + y0 + Rw, 2 : W + 2 : 2],
                                        e1[:],
                                        mybir.ActivationFunctionType.Relu,
                                        bias=bst[:, tower_idx, lay, co, :],
                                    )

                    def stem_layer(tower_dir, tower_wino, tower_idx, lay, src_of, dst_of):
                        if Rw is None:
                            stem_layer_direct(tower_dir, tower_idx, lay, src_of, dst_of)
                        else:
                            stem_layer_wino(tower_wino, tower_idx, lay, src_of, dst_of)

                    # --- cls tower: x->A->B->A->B ---
                    ping = {0: xb, 1: ab, 2: bb, 3: ab}
                    pong = {0: ab, 1: bb, 2: ab, 3: bb}
                    for lay in range(SDEPTH):
                        stem_layer(w_cls, w_cls_w, 0, lay, ping[lay], pong[lay])

                    # --- cls pred: B -> sb_cls (bias, no relu) ---
                    wsl_pc = [[wpc[:, ci, ts(t, NCLS)] for t in range(9)] for ci in range(2)]
                    for g in imgs:
                        for blk in range(nblk):
                            y0 = blk * R
                            psc = pp.tile([NCLS, N], F32, name="psc", tag="ws1", bufs=2)
                            _conv_block(nc, psc, bb[g], wsl_pc, y0, R, W)
                            nc.scalar.activation(
                                sbc[g][:, y0 * W : y0 * W + N],
                                psc[:],
                                mybir.ActivationFunctionType.Identity,
                                bias=bp_cls[:],
                            )

                    # --- box tower: x->A->x->A->x ---
                    bping = {0: xb, 1: ab, 2: xb, 3: ab}
                    bpong = {0: ab, 1: xb, 2: ab, 3: xb}
                    for lay in range(SDEPTH):
                        stem_layer(w_box, w_box_w, 1, lay, bping[lay], bpong[lay])

                    # --- box+ctr pred from x; assemble + write output ---
                    wsl_pb = [[wpb[:, ci, ts(t, 5)] for t in range(9)] for ci in range(2)]
                    for g in imgs:
                        for blk in range(nblk):
                            y0 = blk * R
                            psb = pp.tile([5, N], F32, name="psb", tag="ws2", bufs=2)
                            _conv_block(nc, psb, xb[g], wsl_pb, y0, R, W)
                            sbb = stage.tile([5, N], F32, name="sbb", tag="sbb", bufs=2)
                            nc.scalar.activation(
                                sbb[:],
                                psb[:],
                                mybir.ActivationFunctionType.Identity,
                                bias=bp_bc[:],
                            )
                            for c0 in range(0, N, 128):
                                s0 = y0 * W + c0
                                pst = pp.tile([128, 85], F32, name="pst", tag="ws3", bufs=2)
                                nc.tensor.transpose(
                                    pst[:, 0:NCLS],
                                    sbc[g][:, s0 : s0 + 128],
                                    ident[0:NCLS, 0:NCLS],
                                )
                                nc.tensor.transpose(
                                    pst[:, NCLS:85],
                                    sbb[:, c0 : c0 + 128],
                                    ident[0:5, 0:5],
                                )
                                osb = stage.tile([128, 85], F32, name="osb", tag="osb", bufs=4)
                                nc.scalar.activation(
                                    osb[:], pst[:], mybir.ActivationFunctionType.Copy
                                )
                                nc.sync.dma_start(
                                    out=out_dram[g, base + s0 : base + s0 + 128, :],
                                    in_=osb[:],
                                )
    return nc


def prep_weights(inputs):
    """Host-side reshape of conv weights into lhsT ([ci, co] per tap) layouts."""

    def stem(w):  # (S, O=256, I=256, 3, 3) -> (S, ci_t 2, ci 128, co_t*tap*co)
        S = w.shape[0]
        t = w.transpose(0, 2, 3, 4, 1)  # (S, I, ky, kx, O)
        t = t.reshape(S, 2, 128, 9, 2, 128)  # (S, ci_t, ci, tap, co_t, co)
        t = t.transpose(0, 1, 2, 4, 3, 5)  # (S, ci_t, ci, co_t, tap, co)
        return np.ascontiguousarray(t.reshape(S, 2, 128, 2 * 9 * 128))

    def pred(w):  # (O, 256, 3, 3) -> (ci_t 2, ci 128, tap*O)
        O = w.shape[0]
        t = w.transpose(1, 2, 3, 0)  # (I, ky, kx, O)
        t = t.reshape(2, 128, 9, O)
        return np.ascontiguousarray(t.reshape(2, 128, 9 * O))

    def stem_wino(w):  # (S, O, I, 3, 3) -> (S, ci_t, ci, (co_t i4 dy3 co128))
        S = w.shape[0]
        G = np.array([[1, 0, 0], [0.5, 0.5, 0.5], [0.5, -0.5, 0.5], [0, 0, 1]], np.float64)
        U = np.einsum("xk,soidk->soixd", G, w.astype(np.float64))  # (S,O,I,4,3)
        t = U.transpose(0, 2, 3, 4, 1)  # (S, I, i4, dy3, O)
        t = t.reshape(S, 2, 128, 4, 3, 2, 128)  # (S, ci_t, ci, i, dy, co_t, co)
        t = t.transpose(0, 1, 2, 5, 3, 4, 6)  # (S, ci_t, ci, co_t, i, dy, co)
        return np.ascontiguousarray(t.reshape(S, 2, 128, 2 * 4 * 3 * 128))

    wm = {}
    wm["w_cls"] = stem(inputs["stem_cls_w"]).astype(np.float16)
    wm["w_box"] = stem(inputs["stem_box_w"]).astype(np.float16)
    wm["w_cls_w"] = stem_wino(inputs["stem_cls_w"]).astype(np.float16)
    wm["w_box_w"] = stem_wino(inputs["stem_box_w"]).astype(np.float16)
    wm["w_pcls"] = pred(inputs["pred_cls_w"]).astype(np.float16)
    wm["w_pbc"] = pred(
        np.concatenate([inputs["pred_box_w"], inputs["pred_ctr_w"]], axis=0)
    ).astype(np.float16)
    # stem biases: (S, 256) per tower -> [128, (tower, layer, co_t)]
    bs = np.stack([inputs["stem_cls_b"], inputs["stem_box_b"]], axis=0)  # (2, S, 256)
    bs = bs.reshape(2, SDEPTH, 2, 128).transpose(3, 0, 1, 2)  # (128, 2, S, 2)
    wm["b_stem"] = np.ascontiguousarray(bs.reshape(128, 2 * SDEPTH * 2))
    wm["b_pred"] = np.concatenate(
        [inputs["pred_cls_b"], inputs["pred_box_b"], inputs["pred_ctr_b"]]
    ).reshape(85, 1)
    return {
        k: v if v.dtype == np.float16 else v.astype(np.float32) for k, v in wm.items()
    }


_NC_CACHE = None


def _get_nc():
    global _NC_CACHE
    if _NC_CACHE is None:
        _NC_CACHE = build_nc()
    return _NC_CACHE


def run(inputs, **spmd_kwargs):
    inputs = {k: np.asarray(v) for k, v in inputs.items()}
    nc = _get_nc()
    if not nc.is_finalized():
        nc.finalize()
    wm = prep_weights(inputs)
    feats = [inputs["feat_p3"], inputs["feat_p4"], inputs["feat_p5"]]
    in_maps = []
    for core in range(N_CORES):
        m = dict(wm)
        sl = slice(core * BS, (core + 1) * BS)
        for li in range(3):
            f = feats[li][sl]
            fp = np.zeros(
                (f.shape[0], f.shape[1], f.shape[2] + 2, f.shape[3] + 2), np.float16
            )
            fp[:, :, 1:-1, 1:-1] = f
            m[f"x_l{li}"] = fp
        in_maps.append(m)
    res = run_bass_kernel_spmd(nc, in_maps, list(range(N_CORES)), **spmd_kwargs)
    out = np.concatenate([res.results[i]["out"] for i in range(N_CORES)], axis=0)
    return out, res


def kernel(**inputs):
    return run(inputs)[0]



# revision 4
# speedup vs baseline: 1.1629x; 1.1629x over previous
"""FCOS detection head on 8 Trainium2 NeuronCores (Bass/Tile).

Data parallel: batch 16 -> 2 images per core. Weights replicated.

v2: fp8 (e4m3) DoubleRow matmuls for the p3/p4 stems and preds.
  - DoubleRow contracts K=256 (both ci tiles) per instruction at ~2 rows/cyc,
    ~1.5x the bf16/fp16 PE rate at FD>=256.
  - p3/p4 activations are stored as zero-padded fp8 tiles with each row
    split into parity half-planes [P0(evens) | P1(odds)] so the Winograd
    F(2,3) input transform reads are contiguous.
  - The last stem layer of each tower writes plain raster layout so the
    pred convs can run DoubleRow with flat row-collapsed rhs slices
    (outputs computed over the padded width; garbage border cols dropped
    in the epilogue).
  - All fp8 tensors carry power-of-2 scales (SX per layer, SW for weights),
    folded into the ACT epilogue scale/bias.
  - Winograd output combine: ACT copies m1/m2 to fp16, DVE does the
    adds (fp16 2x mode) plus two scalar_tensor_tensor ops reading PSUM.
  - p5 (16x16) stays on the fp16 direct-conv path from v1.
"""

import numpy as np
import ml_dtypes

import concourse.bacc as bacc
import concourse.bass as bass
import concourse.mybir as mybir
import concourse.tile as tile
from concourse.bass import ts
from concourse.bass_utils import run_bass_kernel_spmd
from concourse.masks import make_identity

F32 = mybir.dt.float32
F16 = mybir.dt.float16
F8 = mybir.dt.float8e4
DR = mybir.MatmulPerfMode.DoubleRow
ADD = mybir.AluOpType.add
SUB = mybir.AluOpType.subtract
MULT = mybir.AluOpType.mult
RELU = mybir.ActivationFunctionType.Relu
COPY = mybir.ActivationFunctionType.Copy
IDENT = mybir.ActivationFunctionType.Identity

N_CORES = 8
B_FULL = 16
BS = B_FULL // N_CORES
C = 256
NCLS = 80
SDEPTH = 4
TAPS = [(ky, kx) for ky in range(3) for kx in range(3)]

FP8MAX = 240.0  # TRN fp8e4 tops out at 240 (not OCP's 448)
SW = 512.0  # fp8 weight scale (stems + preds)
SX = [4.0, 16.0, 32.0, 128.0, 256.0]  # act scales: input, after lay0..lay3

# Winograd levels (p3, p4): H=W, parity-packed padded fp8 activations.
WLEVELS = [
    dict(H=64, Rw=16, base=0,
         pred_blocks=[(0, 7), (7, 7), (14, 7), (21, 7), (28, 7), (35, 7),
                      (42, 7), (49, 7), (56, 7), (63, 1)]),
    dict(H=32, Rw=32, base=4096, pred_blocks=[(0, 15), (15, 15), (30, 2)]),
]
HW_TOTAL = 64 * 64 + 32 * 32 + 16 * 16  # 5376
P5 = dict(H=16, W=16, R=16, base=5120)


def _conv_block(nc, psum, src, wslices, y0, R, W, start_clear=True):
    """fp16 direct conv: 18 accumulating matmuls (p5 path)."""
    n_ci = len(wslices)
    for ci in range(n_ci):
        for t, (dy, dx) in enumerate(TAPS):
            rhs = src[:, ci, y0 + dy : y0 + dy + R, dx : dx + W]
            nc.tensor.matmul(
                psum[:],
                wslices[ci][t],
                rhs,
                start=(start_clear and ci == 0 and t == 0),
                stop=(ci == n_ci - 1 and t == len(TAPS) - 1),
            )


def build_nc():
    nc = bacc.Bacc()

    # --- DRAM parameters (per-core views) ---
    x_dram = {}
    for i, lvl in enumerate(WLEVELS):
        H = lvl["H"]
        # parity-packed padded fp8: cols = [P0 (evens, W/2+1) | P1 (odds)]
        x_dram[i] = nc.declare_dram_parameter(
            f"x_l{i}", [BS, C, H + 2, H + 2], F8, isOutput=False
        )
    x_dram5 = nc.declare_dram_parameter("x_l2", [BS, C, 18, 18], F16, isOutput=False)

    # Winograd F(2,3) transformed fp8 stem weights: cols (co_t, i4, dy3, co128)
    w_cls_w = nc.declare_dram_parameter("w_cls_w", [SDEPTH, 128, 2, 2 * 4 * 3 * 128], F8, isOutput=False)
    w_box_w = nc.declare_dram_parameter("w_box_w", [SDEPTH, 128, 2, 2 * 4 * 3 * 128], F8, isOutput=False)
    # fp8 pred weights: [128, tap9, ci_t2, M]
    w_pcls = nc.declare_dram_parameter("w_pcls", [128, 9, 2, NCLS], F8, isOutput=False)
    w_pbc = nc.declare_dram_parameter("w_pbc", [128, 9, 2, 16], F8, isOutput=False)
    # p5 fp16 weights (v1 layouts)
    w_cls5 = nc.declare_dram_parameter("w_cls5", [SDEPTH, 2, 128, 2 * 9 * 128], F16, isOutput=False)
    w_box5 = nc.declare_dram_parameter("w_box5", [SDEPTH, 2, 128, 2 * 9 * 128], F16, isOutput=False)
    w_pcls5 = nc.declare_dram_parameter("w_pcls5", [2, 128, 9 * NCLS], F16, isOutput=False)
    w_pbc5 = nc.declare_dram_parameter("w_pbc5", [2, 128, 9 * 5], F16, isOutput=False)
    # biases: scaled (for fp8 path) and raw (p5)
    b_stem = nc.declare_dram_parameter("b_stem", [128, 2 * SDEPTH * 2], F32, isOutput=False)
    b_stem5 = nc.declare_dram_parameter("b_stem5", [128, 2 * SDEPTH * 2], F32, isOutput=False)
    b_pred = nc.declare_dram_parameter("b_pred", [85, 1], F32, isOutput=False)
    out_dram = nc.declare_dram_parameter("out", [BS, HW_TOTAL, 85], F32, isOutput=True)

    with tile.TileContext(nc) as tc:
        with (
            tc.tile_pool(name="const", bufs=1) as const,
            tc.tile_pool(name="wp", bufs=3) as wp,
            tc.tile_pool(name="acts", bufs=1) as acts,
            tc.tile_pool(name="stage", bufs=1) as stage,
            tc.tile_pool(name="pp", bufs=1, space="PSUM") as pp,
        ):
            # constants
            ident = const.tile([128, 128], F32, name="ident")
            make_identity(nc, ident[:])
            bst = const.tile([128, 2, SDEPTH, 2, 1], F32, name="bst")
            nc.sync.dma_start(out=bst[:, :, :, :, 0], in_=b_stem[:].rearrange("p (t l c) -> p t l c", t=2, l=SDEPTH, c=2))
            bst5 = const.tile([128, 2, SDEPTH, 2, 1], F32, name="bst5")
            nc.sync.dma_start(out=bst5[:, :, :, :, 0], in_=b_stem5[:].rearrange("p (t l c) -> p t l c", t=2, l=SDEPTH, c=2))
            bp_cls = const.tile([NCLS, 1], F32, name="bp_cls")
            nc.sync.dma_start(out=bp_cls[:], in_=b_pred[0:NCLS])
            bp_bc = const.tile([5, 1], F32, name="bp_bc")
            nc.sync.dma_start(out=bp_bc[:], in_=b_pred[NCLS : NCLS + 5])
            wpc = const.tile([128, 9, 2, NCLS], F8, name="wpc")
            nc.sync.dma_start(out=wpc[:], in_=w_pcls[:])
            wpb = const.tile([128, 9, 2, 16], F8, name="wpb")
            nc.sync.dma_start(out=wpb[:], in_=w_pbc[:])
            wpc5 = const.tile([128, 2, 9 * NCLS], F16, name="wpc5")
            wpb5 = const.tile([128, 2, 9 * 5], F16, name="wpb5")
            for t in range(2):
                nc.sync.dma_start(out=wpc5[:, t, :], in_=w_pcls5[t])
                nc.sync.dma_start(out=wpb5[:, t, :], in_=w_pbc5[t])

            pscale = float(1.0 / (SW * SX[4]))

            # ---------------- p3 / p4: fp8 DoubleRow Winograd ----------------
            for li, lvl in enumerate(WLEVELS):
                H = W = lvl["H"]
                Rw, base = lvl["Rw"], lvl["base"]
                HP, WP = H + 2, W + 2
                Wh, Wh1 = W // 2, W // 2 + 1
                nblk = H // Rw

                for g in range(BS):
                    # activation tiles: [128, 2ci, HP+1(extra pad row), WP] fp8
                    xb = acts.tile([128, 2, HP + 1, WP], F8, name=f"xb{g}", tag=f"xb{g}")
                    ab = acts.tile([128, 2, HP + 1, WP], F8, name=f"ab{g}", tag=f"ab{g}")
                    bb = acts.tile([128, 2, HP + 1, WP], F8, name=f"bb{g}", tag=f"bb{g}")
                    for buf in (ab, bb):
                        nc.gpsimd.memset(buf[:, :, 0, :], 0.0)
                        nc.gpsimd.memset(buf[:, :, H + 1 : H + 3, :], 0.0)
                        nc.gpsimd.memset(buf[:, :, 1 : H + 1, 0], 0.0)
                        nc.gpsimd.memset(buf[:, :, 1 : H + 1, WP - 1], 0.0)
                    nc.gpsimd.memset(xb[:, :, HP, :], 0.0)
                    hh = HP // 2
                    for t in range(2):
                        nc.sync.dma_start(out=xb[:, t, 0:hh, :], in_=x_dram[li][g, ts(t, 128), 0:hh])
                        nc.sync.dma_start(out=xb[:, t, hh:HP, :], in_=x_dram[li][g, ts(t, 128), hh:HP])

                    def stem_layer(tower_w, tower_idx, lay, src, dst, last):
                        wt = wp.tile([128, 2, 2 * 4 * 3 * 128], F8, name="wt", tag="wt")
                        nc.sync.dma_start(out=wt[:], in_=tower_w[lay])
                        scale = float(SX[lay + 1] / (SW * SX[lay]))
                        pv = src.rearrange("p c h (two w) -> p c h two w", two=2, w=Wh1)
                        dpv = dst.rearrange("p c h (two w) -> p c h two w", two=2, w=Wh1)
                        for blk in range(nblk):
                            y0 = blk * Rw
                            vt = [
                                stage.tile([128, 2, Rw + 2, Wh], F8, name=f"vt{i}", tag=f"vt{i}", bufs=3)
                                for i in range(4)
                            ]
                            # (i, a_par, a_off, b_par, b_off, op): vt_i = d_a op d_b
                            vdef = [
                                (0, 0, 0, 0, 1, SUB),  # d0 - d2
                                (1, 1, 0, 0, 1, ADD),  # d1 + d2
                                (2, 0, 1, 1, 0, SUB),  # d2 - d1
                                (3, 1, 0, 1, 1, SUB),  # d1 - d3
                            ]
                            for i, ap_, ao, bp_, bo, op in vdef:
                                for ci in range(2):
                                    rows = pv[:, ci, y0 : y0 + Rw + 2]
                                    nc.vector.tensor_tensor(
                                        vt[i][:, ci],
                                        rows[:, :, ap_, ao : ao + Wh],
                                        rows[:, :, bp_, bo : bo + Wh],
                                        op,
                                    )
                            for co in range(2):
                                ps = [
                                    pp.tile([128, Rw, Wh], F32, name=f"ws{i}", tag=f"ws{i}", bufs=2)
                                    for i in range(4)
                                ]
                                for i in range(4):
                                    for dy in range(3):
                                        nc.tensor.matmul(
                                            ps[i][:],
                                            wt[:, :, ts((co * 4 + i) * 3 + dy, 128)],
                                            vt[i][:, :, dy : dy + Rw, :],
                                            start=(dy == 0),
                                            stop=(dy == 2),
                                            perf_mode=DR,
                                        )
                                # e0 = m0+m1+m2, e1 = m1-m2-m3
                                c1 = stage.tile([128, Rw, Wh], F16, name="c1", tag="c1", bufs=2)
                                c2 = stage.tile([128, Rw, Wh], F16, name="c2", tag="c2", bufs=2)
                                t0 = stage.tile([128, Rw, Wh], F16, name="t0", tag="t0", bufs=2)
                                uu = stage.tile([128, Rw, Wh], F16, name="uu", tag="uu", bufs=2)
                                e0 = stage.tile([128, Rw, Wh], F16, name="e0", tag="e0", bufs=2)
                                e1 = stage.tile([128, Rw, Wh], F16, name="e1", tag="e1", bufs=2)
                                nc.scalar.activation(c1[:], ps[1][:], COPY)
                                nc.scalar.activation(c2[:], ps[2][:], COPY)
                                nc.vector.tensor_tensor(t0[:], c1[:], c2[:], ADD)
                                nc.vector.tensor_tensor(uu[:], c1[:], c2[:], SUB)
                                nc.vector.scalar_tensor_tensor(e0[:], ps[0][:], 1.0, t0[:], MULT, ADD)
                                nc.vector.scalar_tensor_tensor(e1[:], ps[3][:], -1.0, uu[:], MULT, ADD)
                                bias = bst[:, tower_idx, lay, co, :]
                                if not last:
                                    # e0 -> even out cols (P1[0:Wh]); e1 -> odd (P0[1:Wh1])
                                    nc.scalar.activation(
                                        dpv[:, co, 1 + y0 : 1 + y0 + Rw, 1, 0:Wh],
                                        e0[:], RELU, bias=bias, scale=scale)
                                    nc.scalar.activation(
                                        dpv[:, co, 1 + y0 : 1 + y0 + Rw, 0, 1:Wh1],
                                        e1[:], RELU, bias=bias, scale=scale)
                                else:
                                    # plain raster layout for the pred convs
                                    nc.scalar.activation(
                                        dst[:, co, 1 + y0 : 1 + y0 + Rw, 1 : W + 1 : 2],
                                        e0[:], RELU, bias=bias, scale=scale)
                                    nc.scalar.activation(
                                        dst[:, co, 1 + y0 : 1 + y0 + Rw, 2 : W + 2 : 2],
                                        e1[:], RELU, bias=bias, scale=scale)

                    # cls tower: x->A->B->A->B ; box tower: x->A->x->A->x
                    ping = {0: xb, 1: ab, 2: bb, 3: ab}
                    pong = {0: ab, 1: bb, 2: ab, 3: bb}
                    for lay in range(SDEPTH):
                        stem_layer(w_cls_w, 0, lay, ping[lay], pong[lay], lay == SDEPTH - 1)
                    bping = {0: xb, 1: ab, 2: xb, 3: ab}
                    bpong = {0: ab, 1: xb, 2: ab, 3: xb}
                    for lay in range(SDEPTH):
                        stem_layer(w_box_w, 1, lay, bping[lay], bpong[lay], lay == SDEPTH - 1)

                    # ---- preds: DoubleRow direct conv over padded width ----
                    sbc = stage.tile([NCLS, H * W], F32, name="sbc", tag=f"sbc{g}")
                    sbb = stage.tile([5, H * W], F32, name="sbb", tag=f"sbb{g}")
                    bbf = bb.rearrange("p c h w -> p c (h w)")
                    xbf = xb.rearrange("p c h w -> p c (h w)")
                    for (y0, R) in lvl["pred_blocks"]:
                        Np = R * WP
                        psc = pp.tile([NCLS, Np], F32, name="psc", tag="ws1", bufs=2)
                        for t, (dy, dx) in enumerate(TAPS):
                            off = (y0 + dy) * WP + dx
                            nc.tensor.matmul(
                                psc[:], wpc[:, t], bbf[:, :, off : off + Np],
                                start=(t == 0), stop=(t == 8), perf_mode=DR)
                        nc.scalar.activation(
                            sbc[:, y0 * W : (y0 + R) * W].rearrange("p (r w) -> p r w", w=W),
                            psc[:].rearrange("p (r w) -> p r w", w=WP)[:, :, 0:W],
                            IDENT, bias=bp_cls[:], scale=pscale)
                        psb = pp.tile([16, Np], F32, name="psb", tag="ws2", bufs=2)
                        for t, (dy, dx) in enumerate(TAPS):
                            off = (y0 + dy) * WP + dx
                            nc.tensor.matmul(
                                psb[:], wpb[:, t], xbf[:, :, off : off + Np],
                                start=(t == 0), stop=(t == 8), perf_mode=DR)
                        nc.scalar.activation(
                            sbb[:, y0 * W : (y0 + R) * W].rearrange("p (r w) -> p r w", w=W),
                            psb[0:5].rearrange("p (r w) -> p r w", w=WP)[:, :, 0:W],
                            IDENT, bias=bp_bc[:], scale=pscale)

                    # ---- assemble + write output ----
                    for c0 in range(0, H * W, 128):
                        pst = pp.tile([128, 85], F32, name="pst", tag="ws3", bufs=2)
                        nc.tensor.transpose(pst[:, 0:NCLS], sbc[:, c0 : c0 + 128], ident[0:NCLS, 0:NCLS])
                        nc.tensor.transpose(pst[:, NCLS:85], sbb[:, c0 : c0 + 128], ident[0:5, 0:5])
                        osb = stage.tile([128, 85], F32, name="osb", tag="osb", bufs=4)
                        nc.scalar.activation(osb[:], pst[:], COPY)
                        nc.sync.dma_start(out=out_dram[g, base + c0 : base + c0 + 128, :], in_=osb[:])

            # ---------------- p5: fp16 direct (v1 path) ----------------
            H = W = P5["H"]
            R, base = P5["R"], P5["base"]
            HP, WP = H + 2, W + 2
            N = R * W
            xb5, ab5, bb5, sbc5 = {}, {}, {}, {}
            for g in range(BS):
                xb5[g] = acts.tile([128, 2, HP, WP], F16, name=f"x5b{g}", tag=f"x5b{g}")
                ab5[g] = acts.tile([128, 2, HP, WP], F16, name=f"a5b{g}", tag=f"a5b{g}")
                bb5[g] = acts.tile([128, 2, HP, WP], F16, name=f"b5b{g}", tag=f"b5b{g}")
                sbc5[g] = stage.tile([NCLS, H * W], F32, name=f"s5c{g}", tag=f"s5c{g}")
                for buf in (ab5[g], bb5[g]):
                    nc.gpsimd.memset(buf[:, :, 0, :], 0.0)
                    nc.gpsimd.memset(buf[:, :, H + 1, :], 0.0)
                    nc.gpsimd.memset(buf[:, :, 1 : H + 1, 0], 0.0)
                    nc.gpsimd.memset(buf[:, :, 1 : H + 1, W + 1], 0.0)
                for t in range(2):
                    nc.sync.dma_start(out=xb5[g][:, t, :, :], in_=x_dram5[g, ts(t, 128)])

            def stem_layer5(tower_w, tower_idx, lay, src_of, dst_of):
                wt = wp.tile([128, 2, 2 * 9 * 128], F16, name="wt5", tag="wt5")
                for t in range(2):
                    nc.sync.dma_start(out=wt[:, t, :], in_=tower_w[lay, t])
                for g in range(BS):
                    src, dst = src_of[g], dst_of[g]
                    for co in range(2):
                        ps = pp.tile([128, N], F32, name="ps", tag="ws0", bufs=2)
                        wsl_co = [
                            [wt[:, ci, ts(co * 9 + t, 128)] for t in range(9)]
                            for ci in range(2)
                        ]
                        _conv_block(nc, ps, src, wsl_co, 0, R, W)
                        nc.scalar.activation(
                            dst[:, co, 1 : 1 + R, 1 : 1 + W],
                            ps[:].rearrange("p (r w) -> p r w", w=W),
                            RELU,
                            bias=bst5[:, tower_idx, lay, co, :],
                        )

            ping = {0: xb5, 1: ab5, 2: bb5, 3: ab5}
            pong = {0: ab5, 1: bb5, 2: ab5, 3: bb5}
            for lay in range(SDEPTH):
                stem_layer5(w_cls5, 0, lay, ping[lay], pong[lay])

            wsl_pc = [[wpc5[:, ci, ts(t, NCLS)] for t in range(9)] for ci in range(2)]
            for g in range(BS):
                psc = pp.tile([NCLS, N], F32, name="psc5", tag="ws1", bufs=2)
                _conv_block(nc, psc, bb5[g], wsl_pc, 0, R, W)
                nc.scalar.activation(sbc5[g][:], psc[:], IDENT, bias=bp_cls[:])

            bping = {0: xb5, 1: ab5, 2: xb5, 3: ab5}
            bpong = {0: ab5, 1: xb5, 2: ab5, 3: xb5}
            for lay in range(SDEPTH):
                stem_layer5(w_box5, 1, lay, bping[lay], bpong[lay])

            wsl_pb = [[wpb5[:, ci, ts(t, 5)] for t in range(9)] for ci in range(2)]
            for g in range(BS):
                psb = pp.tile([5, N], F32, name="psb5", tag="ws2", bufs=2)
                _conv_block(nc, psb, xb5[g], wsl_pb, 0, R, W)
                sbb = stage.tile([5, N], F32, name="sbb5", tag="sbb5", bufs=2)
                nc.scalar.activation(sbb[:], psb[:], IDENT, bias=bp_bc[:])
                for c0 in range(0, N, 128):
                    pst = pp.tile([128, 85], F32, name="pst5", tag="ws3", bufs=2)
                    nc.tensor.transpose(pst[:, 0:NCLS], sbc5[g][:, c0 : c0 + 128], ident[0:NCLS, 0:NCLS])
                    nc.tensor.transpose(pst[:, NCLS:85], sbb[:, c0 : c0 + 128], ident[0:5, 0:5])
                    osb = stage.tile([128, 85], F32, name="osb5", tag="osb", bufs=4)
                    nc.scalar.activation(osb[:], pst[:], COPY)
                    nc.sync.dma_start(out=out_dram[g, base + c0 : base + c0 + 128, :], in_=osb[:])
    return nc


E4 = ml_dtypes.float8_e4m3


def _q8(a):
    return np.clip(a, -FP8MAX, FP8MAX).astype(E4)


def prep_weights(inputs):
    """Host-side weight transforms/quantization."""
    S = SDEPTH

    def stem_wino8(w):  # (S, O, I, 3, 3) -> [S, 128, 2, (co_t i dy co)] fp8
        G = np.array([[1, 0, 0], [0.5, 0.5, 0.5], [0.5, -0.5, 0.5], [0, 0, 1]], np.float64)
        U = np.einsum("xk,soidk->soixd", G, w.astype(np.float64))  # (S,O,I,4dy? no: x over kx)
        t = U.transpose(0, 2, 1, 3, 4)  # (S, I, O, i4, dy3)
        t = t.reshape(S, 2, 128, 2, 128, 4, 3)  # (S, It, Ii, Ot, Oi, i, dy)
        t = t.transpose(0, 2, 1, 3, 5, 6, 4)  # (S, Ii, It, Ot, i, dy, Oi)
        return _q8(SW * t.reshape(S, 128, 2, 2 * 4 * 3 * 128))

    def pred8(w, pad_to):  # (O, 256, 3, 3) -> [128, 9, 2, pad_to] fp8
        O = w.shape[0]
        t = w.transpose(1, 2, 3, 0)  # (I, ky, kx, O)
        t = t.reshape(2, 128, 9, O).transpose(1, 2, 0, 3)  # (Ii, tap, It, O)
        if pad_to > O:
            t = np.pad(t, [(0, 0), (0, 0), (0, 0), (0, pad_to - O)])
        return _q8(SW * t)

    def stem16(w):  # v1 fp16 layout for p5
        t = w.transpose(0, 2, 3, 4, 1)
        t = t.reshape(S, 2, 128, 9, 2, 128)
        t = t.transpose(0, 1, 2, 4, 3, 5)
        return np.ascontiguousarray(t.reshape(S, 2, 128, 2 * 9 * 128)).astype(np.float16)

    def pred16(w):
        O = w.shape[0]
        t = w.transpose(1, 2, 3, 0)
        t = t.reshape(2, 128, 9, O)
        return np.ascontiguousarray(t.reshape(2, 128, 9 * O)).astype(np.float16)

    wm = {}
    wm["w_cls_w"] = stem_wino8(np.asarray(inputs["stem_cls_w"]))
    wm["w_box_w"] = stem_wino8(np.asarray(inputs["stem_box_w"]))
    wm["w_pcls"] = pred8(np.asarray(inputs["pred_cls_w"]), NCLS)
    wm["w_pbc"] = pred8(
        np.concatenate([inputs["pred_box_w"], inputs["pred_ctr_w"]], axis=0), 16)
    wm["w_cls5"] = stem16(np.asarray(inputs["stem_cls_w"]))
    wm["w_box5"] = stem16(np.asarray(inputs["stem_box_w"]))
    wm["w_pcls5"] = pred16(np.asarray(inputs["pred_cls_w"]))
    wm["w_pbc5"] = pred16(
        np.concatenate([inputs["pred_box_w"], inputs["pred_ctr_w"]], axis=0))
    # stem biases: (tower2, S, 256) -> [128, (tower, layer, co_t)]
    bs = np.stack([inputs["stem_cls_b"], inputs["stem_box_b"]], axis=0).astype(np.float64)
    bss = bs * np.asarray(SX[1:], np.float64)[None, :, None]  # pre-scaled per layer
    def blayout(b):
        return np.ascontiguousarray(
            b.reshape(2, SDEPTH, 2, 128).transpose(3, 0, 1, 2).reshape(128, 2 * SDEPTH * 2)
        ).astype(np.float32)
    wm["b_stem"] = blayout(bss)
    wm["b_stem5"] = blayout(bs)
    wm["b_pred"] = np.concatenate(
        [inputs["pred_cls_b"], inputs["pred_box_b"], inputs["pred_ctr_b"]]
    ).reshape(85, 1).astype(np.float32)
    return wm


_NC_CACHE = None


def _get_nc():
    global _NC_CACHE
    if _NC_CACHE is None:
        _NC_CACHE = build_nc()
    return _NC_CACHE


def run(inputs, **spmd_kwargs):
    inputs = {k: np.asarray(v) for k, v in inputs.items()}
    nc = _get_nc()
    if not nc.is_finalized():
        nc.finalize()
    wm = prep_weights(inputs)
    feats = [inputs["feat_p3"], inputs["feat_p4"], inputs["feat_p5"]]
    in_maps = []
    for core in range(N_CORES):
        m = dict(wm)
        sl = slice(core * BS, (core + 1) * BS)
        for li in range(2):
            f = np.asarray(feats[li][sl], np.float32) * SX[0]
            B_, C_, H_, W_ = f.shape
            fp = np.zeros((B_, C_, H_ + 2, W_ + 2), np.float32)
            fp[:, :, 1:-1, 1:-1] = f
            # parity pack: cols -> [P0 (evens) | P1 (odds)]
            fpp = np.concatenate([fp[..., 0::2], fp[..., 1::2]], axis=-1)
            m[f"x_l{li}"] = _q8(fpp)
        f = np.asarray(feats[2][sl], np.float32)
        fp = np.zeros((f.shape[0], f.shape[1], 18, 18), np.float16)
        fp[:, :, 1:-1, 1:-1] = f
        m["x_l2"] = fp
        in_maps.append(m)
    res = run_bass_kernel_spmd(nc, in_maps, list(range(N_CORES)), **spmd_kwargs)
    out = np.concatenate([res.results[i]["out"] for i in range(N_CORES)], axis=0)
    return out, res


def kernel(**inputs):
    return run(inputs)[0]
